# revision 1
# baseline (speedup 1.0000x reference)
"""MultiHeadAttention with softmax over the HEAD axis — TRN2 Bass kernel.

Reference computation (E=1024, H=16, D=64, B=2, S=2048):
    qh = split_heads(q @ Wq.T); kh = split_heads(k @ Wk.T); vh = split_heads(v @ Wv.T)
    scores = einsum("bhqd,bhkd->bhqk", qh, kh) / 8
    attn = softmax(scores, axis=1)            # over HEADS, not keys
    out = merge_heads(einsum("bhqk,bhkd->bhqd", attn, vh)) @ Wo.T

Sharding: 8 cores = 2 batches x 4 query-blocks of 512. Each core computes
K/V projections for its whole batch (replicated within the 4-core group),
Q projection for its 512 queries, the head-softmax attention, and the output
projection for its query block. No collectives.

On-chip layout (all matmul operands bf16, PSUM f32):
    qhT  [e_out, q]  8 tiles [128, 512]    kh/qh head h = partitions
    khT  [e_out, j]  8 tiles [128, 2048]     (h%2)*64..+64 of tile h//2
    vh   [j, e_out] 16 tiles [128, 1024]
    scores_T[j, q] per head via matmul(lhsT=khT slice, rhs=qhT slice)
    attn group tile [128, 2jc, 16h, 256q]; softmax over h = exp (ACT) ->
      pairwise tree adds (DVE) -> reciprocal_approx_fast -> broadcast mul
    attnv: out[d-pair 128, q 256] accumulated over j in PSUM (per head M=64)
    out projection: outT[e_out, q] = WoT.T @ mergedT
"""
import sys
sys.path.insert(0, "/opt/trn_rl_repo")

import numpy as np
import ml_dtypes

import concourse.bass as bass
from concourse import bacc
import concourse.mybir as mybir
import concourse.tile as tile
from concourse.bass_utils import run_bass_kernel_spmd

F32 = mybir.dt.float32
BF16 = mybir.dt.bfloat16
AF = mybir.ActivationFunctionType

B, S, E, H, D = 2, 2048, 1024, 16, 64
NCORES = 8
QB = S * B // NCORES          # 512 queries per core
QP = 256                      # query pass size
NPASS = QB // QP              # 2
SK = S                        # 2048 keys
NJC = SK // 128               # 16 j-chunks
GJC = 2                       # j-chunks per attn group
NGRP = NJC // GJC             # 8 groups
NEI = E // 128                # 8 e_in chunks
NEO = E // 128                # 8 e_out chunks

_CACHED = {}


def build():
    nc = bacc.Bacc(trn_type="TRN2", target_bir_lowering=False)

    qT = nc.dram_tensor("qT", [NEI, 128, QB], BF16, kind="ExternalInput")
    kT = nc.dram_tensor("kT", [NEI, 128, SK], BF16, kind="ExternalInput")
    vT = nc.dram_tensor("vT", [NEI, 128, SK], BF16, kind="ExternalInput")
    WqT = nc.dram_tensor("WqT", [NEI, 128, E], BF16, kind="ExternalInput")
    WkT = nc.dram_tensor("WkT", [NEI, 128, E], BF16, kind="ExternalInput")
    WvT = nc.dram_tensor("WvT", [NEI, 128, E], BF16, kind="ExternalInput")
    WoT = nc.dram_tensor("WoT", [NEI, 128, E], BF16, kind="ExternalInput")
    outT = nc.dram_tensor("outT", [NEO, 128, QB], F32, kind="ExternalOutput")

    with tile.TileContext(nc) as tc:
        with tc.tile_pool(name="persist", bufs=1) as persist, \
             tc.tile_pool(name="mm", bufs=2, space="PSUM") as mm, \
             tc.tile_pool(name="avp", bufs=1, space="PSUM") as avp:

            # qhTz[par][t]: rows of head 2t+par live, other 64 rows zero
            qhTz = [[persist.tile([128, QB], BF16, tag=f"qhTz{par}_{i}",
                                  name=f"qhTz{par}_{i}") for i in range(NEO)]
                    for par in range(2)]
            khT = [persist.tile([128, SK], BF16, tag=f"khT{i}", name=f"khT{i}") for i in range(NEO)]
            vh = [persist.tile([128, E], BF16, tag=f"vh{i}", name=f"vh{i}") for i in range(NJC)]

            # weights needed later, prefetched while phase A runs
            wlate = persist
            w_v = [wlate.tile([128, E], BF16, tag=f"wv{i}", name=f"wv{i}") for i in range(NEI)]
            w_o = [wlate.tile([128, E], BF16, tag=f"wo{i}", name=f"wo{i}") for i in range(NEI)]
            for i in range(NEI):
                nc.sync.dma_start(out=w_v[i], in_=WvT[i])
                nc.sync.dma_start(out=w_o[i], in_=WoT[i])

            # ---------------- Phase A1: q and k projections ----------------
            with tc.tile_pool(name="pa", bufs=1) as pa:
                w_q = [pa.tile([128, E], BF16, tag=f"wq{i}", name=f"wq{i}") for i in range(NEI)]
                w_k = [pa.tile([128, E], BF16, tag=f"wk{i}", name=f"wk{i}") for i in range(NEI)]
                x_q = [pa.tile([128, QB], BF16, tag=f"xq{i}", name=f"xq{i}") for i in range(NEI)]
                x_k = [pa.tile([128, SK], BF16, tag=f"xk{i}", name=f"xk{i}") for i in range(NEI)]

                for i in range(NEI):
                    nc.sync.dma_start(out=x_q[i], in_=qT[i])
                    nc.sync.dma_start(out=w_q[i], in_=WqT[i])
                    nc.sync.dma_start(out=x_k[i], in_=kT[i])
                    nc.sync.dma_start(out=w_k[i], in_=WkT[i])

                for par in range(2):
                    for eo in range(NEO):
                        nc.vector.memset(qhTz[par][eo], 0.0)
                for eo in range(NEO):
                    ps = mm.tile([128, 512], F32, tag="ps")
                    for ki in range(NEI):
                        nc.tensor.matmul(ps[:, 0:QB],
                                         w_q[ki][:, eo * 128:(eo + 1) * 128],
                                         x_q[ki],
                                         start=(ki == 0), stop=(ki == NEI - 1))
                    nc.scalar.copy(qhTz[0][eo][0:64, :], ps[0:64, 0:QB])
                    nc.scalar.copy(qhTz[1][eo][64:128, :], ps[64:128, 0:QB])

                # k projection, j-column-major so early j-chunks finish first
                for jn in range(SK // 512):
                    for eo in range(NEO):
                        ps = mm.tile([128, 512], F32, tag="ps")
                        for ki in range(NEI):
                            nc.tensor.matmul(
                                ps,
                                w_k[ki][:, eo * 128:(eo + 1) * 128],
                                x_k[ki][:, jn * 512:(jn + 1) * 512],
                                start=(ki == 0), stop=(ki == NEI - 1))
                        nc.scalar.copy(khT[eo][:, jn * 512:(jn + 1) * 512], ps)

            # ---------------- Phase A2/B/C: v projection + attention ------
            with tc.tile_pool(name="pb", bufs=1) as pb, \
                 tc.tile_pool(name="xvp", bufs=2) as xvp, \
                 tc.tile_pool(name="attnp", bufs=2) as attnp, \
                 tc.tile_pool(name="tmp", bufs=1) as tmp, \
                 tc.tile_pool(name="outp", bufs=2) as outp:
                mergedT = [pb.tile([128, QB], BF16, tag=f"mergedT{i}",
                                   name=f"mergedT{i}") for i in range(NEO)]

                def vh_proj(jn):
                    # v projection for j-chunks 4jn..4jn+3 (j-cols jn*512..)
                    xv = [xvp.tile([128, 512], BF16, tag=f"xv{i}",
                                   name=f"xv{jn}_{i}") for i in range(NEI)]
                    for i in range(NEI):
                        nc.sync.dma_start(out=xv[i],
                                          in_=vT[i][:, jn * 512:(jn + 1) * 512])
                    for jj in range(4):
                        jc = jn * 4 + jj
                        for en in range(E // 512):
                            ps = mm.tile([128, 512], F32, tag="ps")
                            for ki in range(NEI):
                                nc.tensor.matmul(
                                    ps,
                                    xv[ki][:, jj * 128:(jj + 1) * 128],
                                    w_v[ki][:, en * 512:(en + 1) * 512],
                                    start=(ki == 0), stop=(ki == NEI - 1))
                            nc.scalar.copy(vh[jc][:, en * 512:(en + 1) * 512], ps)

                def scores_softmax(p, g):
                    qs = p * QP
                    attn_g = attnp.tile([128, GJC, H, QP], BF16, tag="attn",
                                        name=f"attn{p}_{g}")
                    for jj in range(GJC):
                        jc = g * GJC + jj
                        for hg in range(H // 4):
                            sp = mm.tile([128, 4, QP], F32, tag="ps")
                            for hh in range(4):
                                h = hg * 4 + hh
                                t, par = h // 2, h % 2
                                nc.tensor.matmul(
                                    sp[:, hh, :],
                                    khT[t][:, jc * 128:(jc + 1) * 128],
                                    qhTz[par][t][:, qs:qs + QP],
                                    start=True, stop=True)
                            nc.scalar.activation(
                                attn_g[:, jj, hg * 4:(hg + 1) * 4, :],
                                sp, AF.Exp, scale=0.125)
                    # softmax over h: tree-sum, fast reciprocal, scale
                    t1 = tmp.tile([128, GJC, 8, QP], BF16, tag="t1")
                    nc.vector.tensor_add(t1, attn_g[:, :, 0:8, :],
                                         attn_g[:, :, 8:16, :])
                    t2 = tmp.tile([128, GJC, 4, QP], BF16, tag="t2")
                    nc.vector.tensor_add(t2, t1[:, :, 0:4, :], t1[:, :, 4:8, :])
                    t3 = tmp.tile([128, GJC, 2, QP], BF16, tag="t3")
                    nc.vector.tensor_add(t3, t2[:, :, 0:2, :], t2[:, :, 2:4, :])
                    zf = tmp.tile([128, GJC, QP], F32, tag="zf")
                    nc.vector.tensor_add(zf, t3[:, :, 0, :], t3[:, :, 1, :])
                    rf = tmp.tile([128, GJC, QP], F32, tag="rf")
                    nc.vector.reciprocal_approx_fast(
                        out=rf.rearrange("p a q -> p (a q)"),
                        in_=zf.rearrange("p a q -> p (a q)"))
                    r16 = tmp.tile([128, GJC, QP], BF16, tag="r16")
                    nc.vector.tensor_copy(r16, rf)
                    nc.vector.tensor_mul(
                        attn_g, attn_g,
                        r16.unsqueeze(2).broadcast_to([128, GJC, H, QP]))
                    return attn_g

                def attnv(avt, attn_g, g):
                    for jj in range(GJC):
                        jc = g * GJC + jj
                        for h in range(H):
                            pp, half = h // 2, (h % 2) * 64
                            nc.tensor.matmul(
                                avt[pp // 2][half:half + 64, pp % 2, :],
                                vh[jc][:, h * 64:(h + 1) * 64],
                                attn_g[:, jj, h, :],
                                start=(jc == 0 and h % 4 < 2),
                                stop=(jc == NJC - 1 and h % 4 >= 2),
                                skip_group_check=True)

                for p in range(NPASS):
                    qs = p * QP
                    avt = [avp.tile([128, 2, QP], F32, tag=f"avt{i}",
                                    name=f"avt{p}_{i}") for i in range(4)]
                    pending = None  # (attn_g, g) with attnv not yet emitted
                    for g in range(NGRP):
                        if p == 0 and g % 2 == 0:
                            vh_proj(g // 2)
                        attn_g = scores_softmax(p, g)
                        if pending is not None:
                            attnv(avt, *pending)
                        pending = (attn_g, g)
                    attnv(avt, *pending)
                    for pp in range(8):
                        nc.scalar.copy(mergedT[pp][:, qs:qs + QP],
                                       avt[pp // 2][:, pp % 2, :])

                # ---------------- Phase D: output projection --------------
                for eo in range(NEO):
                    ps = mm.tile([128, 512], F32, tag="ps")
                    for ki in range(NEI):
                        nc.tensor.matmul(ps[:, 0:QB],
                                         w_o[ki][:, eo * 128:(eo + 1) * 128],
                                         mergedT[ki],
                                         start=(ki == 0), stop=(ki == NEI - 1))
                    ot = outp.tile([128, QB], F32, tag="ot")
                    nc.vector.tensor_copy(ot, ps[:, 0:QB])
                    nc.sync.dma_start(out=outT[eo], in_=ot)

    nc.compile()
    return nc


def _get_nc():
    if "nc" not in _CACHED:
        _CACHED["nc"] = build()
    return _CACHED["nc"]


def kernel(q, k, v, Wq, Wk, Wv, Wo, **unused):
    q = np.asarray(q, dtype=np.float32)
    k = np.asarray(k, dtype=np.float32)
    v = np.asarray(v, dtype=np.float32)

    bf = ml_dtypes.bfloat16

    def prep_w(W):
        # [out,in] -> W.T [in,out] -> [NEI, 128, E], bf16
        return np.ascontiguousarray(
            np.asarray(W, dtype=np.float32).T.reshape(NEI, 128, E)
        ).astype(bf)

    WqT, WkT, WvT, WoT = map(prep_w, (Wq, Wk, Wv, Wo))

    kT_b, vT_b = [], []
    for b in range(B):
        kT_b.append(np.ascontiguousarray(
            k[b].T.reshape(NEI, 128, SK)).astype(bf))
        vT_b.append(np.ascontiguousarray(
            v[b].T.reshape(NEI, 128, SK)).astype(bf))

    in_maps = []
    for c in range(NCORES):
        b, qs = c // 4, (c % 4) * QB
        qT_c = np.ascontiguousarray(
            q[b].T[:, qs:qs + QB].reshape(NEI, 128, QB)).astype(bf)
        in_maps.append({
            "qT": qT_c, "kT": kT_b[b], "vT": vT_b[b],
            "WqT": WqT, "WkT": WkT, "WvT": WvT, "WoT": WoT,
        })

    nc = _get_nc()
    res = run_bass_kernel_spmd(nc, in_maps, core_ids=list(range(NCORES)))

    out = np.empty((B, S, E), dtype=np.float32)
    for c in range(NCORES):
        b, qs = c // 4, (c % 4) * QB
        oT = res.results[c]["outT"].reshape(E, QB)
        out[b, qs:qs + QB, :] = oT.T
    return out

